# revision 7
# baseline (speedup 1.0000x reference)
"""Trainium2 Bass kernel for nn_MixtureAlignmentLogLikelihood.

Math: with trg_p = softmax(trg_sent, axis=2), every row of trg_p sums to 1
and P_st is the uniform matrix 1/Kt, so dot[b, t] = 1/Kt exactly and

  log_likelihood = -log(Kt) * sum(scales)

sum(scales) depends only on trg_boundary: per batch row (T positions,
boundary bits z in {0,1}):

  count = popcount(z); first = z[0]; lastp1 = (last set index)+1 (0 if none)
  sum_scales = count - first - max(lastp1, 1) + T + 1

Device kernel (per core): the 32 batch rows are laid out as 128 SBUF
partitions x 512 free (4 chunks of 512 per row, chunk j of row r on
partition 4r+j), processed as two free-dim halves (a = [0,256),
b = [256,512)) so compute on half a overlaps half b's DMA.  Per
partition/half the device computes
  m    = max_i  z[i] * (chunk_local_index(i)+1)   (local lastp1)
  cnt  = sum_i  z[i]                              (popcount)
via DVE mult + max-reduce and a scalar-engine activation add-accumulate
running in parallel.  The [128,4] result (m_a, cnt_a, m_b, cnt_b) is
DMA'd out on both HWDGE queues (64 rows each, parallel descriptor
fetch); the O(B) half/chunk combine plus the scalar all-reduce over
rows/cores happens on the host during the gather, as does reading
first = z[0] directly from the input.  All quantities are small
integers -> exact in int8/int16/f32.

Schedule: no nc.Block() -- instructions are emitted at top level so the
input DMAs are the very first post-preamble instructions on each HWDGE
queue (SP rows 0:64, Activation rows 64:128; half a then half b on each
queue so half a lands first).  The gpsimd iotas and the scalar ACT
table load (hoisted via a dummy activation on a framework const AP)
overlap the input DMA.  Sems: sa/sb gate half readiness (two DMAs + one
iota each, +16 apiece -> wait 48), dn counts the four compute results
(wait 4) and gates the output DMAs.  The final output DMAs are not
engine-waited: NEFF completion semantics (engine halt + DGE queue
quiesce in the runtime) cover them, verified empirically over repeated
randomized runs of previous kernel revisions.
"""

import math

import numpy as np

B, T, K = 256, 2048, 64
N_CORES = 8
BS = B // N_CORES  # 32 batch rows per core
NCHUNK = 4
CH = T // NCHUNK  # 512
HF = CH // 2  # 256, free-dim half
NEG_LOG_K = -math.log(float(K))

_CACHE: dict = {}


def _build_nc(final_wait: bool = False):
    import concourse.bass as bass
    import concourse.mybir as mybir

    f32 = mybir.dt.float32
    i16 = mybir.dt.int16
    i8 = mybir.dt.int8
    Copy = mybir.ActivationFunctionType.Copy

    P = BS * NCHUNK  # 128 partitions
    HP = P // 2  # 64 partitions per DMA queue

    nc = bass.Bass(enable_partition_id=False, monotonic_sem_count=0)
    tb = nc.dram_tensor("tb", [P, CH], i8, kind="ExternalInput")
    out = nc.dram_tensor("out", [P, 4], f32, kind="ExternalOutput")

    with (
        nc.sbuf_tensor("tbs", [P, CH], i8) as tbs,
        nc.sbuf_tensor("iot", [P, CH], i16) as iot,
        nc.sbuf_tensor("prod", [P, CH], i16) as prod,
        nc.sbuf_tensor("adum", [P, CH], i8) as adum,
        nc.sbuf_tensor("dum1", [P, 1], f32) as dum1,
        nc.sbuf_tensor("outb", [P, 4], f32) as outb,
        nc.semaphore("sa") as sa,
        nc.semaphore("sb") as sb,
        nc.semaphore("dn") as dn,
    ):
        c0 = nc.const_aps.aps[(f32, 0.0)]

        # Input DMAs -- first post-preamble instructions on each HWDGE
        # queue; half a (free 0:256) ahead of half b on both queues so
        # compute on a overlaps b's transfer.
        nc.sync.dma_start(tbs[0:HP, 0:HF], tb[0:HP, 0:HF]).then_inc(sa, 16)
        nc.scalar.dma_start(tbs[HP:P, 0:HF], tb[HP:P, 0:HF]).then_inc(sa, 16)
        nc.sync.dma_start(tbs[0:HP, HF:CH], tb[0:HP, HF:CH]).then_inc(sb, 16)
        nc.scalar.dma_start(tbs[HP:P, HF:CH], tb[HP:P, HF:CH]).then_inc(sb, 16)

        # Chunk-local index vectors (+1-based; half b bakes in the +256).
        nc.gpsimd.iota(
            iot[:, 0:HF], pattern=[[1, HF]], base=1, channel_multiplier=0
        ).then_inc(sa, 16)
        nc.gpsimd.iota(
            iot[:, HF:CH], pattern=[[1, HF]], base=1 + HF, channel_multiplier=0
        ).then_inc(sb, 16)

        # Dummy activation on a framework const AP: hoists the 1.3us
        # ACT_TABLE_LOAD into the DMA window instead of after it.
        nc.scalar.activation(dum1[:], c0, Copy)

        # cnt = add-accumulate of Copy(tb); f32 accum of 0/1 ints is exact
        nc.scalar.wait_ge(sa, 48)
        nc.scalar.activation(
            adum[:, 0:HF], tbs[:, 0:HF], Copy, accum_out=outb[:, 1:2]
        ).then_inc(dn, 1)
        nc.scalar.wait_ge(sb, 48)
        nc.scalar.activation(
            adum[:, HF:CH], tbs[:, HF:CH], Copy, accum_out=outb[:, 3:4]
        ).then_inc(dn, 1)

        # m = max_i tb[i]*(local index+1) per half
        nc.vector.wait_ge(sa, 48)
        nc.vector.tensor_tensor(
            prod[:, 0:HF], tbs[:, 0:HF], iot[:, 0:HF], op=mybir.AluOpType.mult
        )
        nc.vector.tensor_reduce(
            outb[:, 0:1], prod[:, 0:HF], axis=mybir.AxisListType.X,
            op=mybir.AluOpType.max,
        ).then_inc(dn, 1)
        nc.vector.wait_ge(sb, 48)
        nc.vector.tensor_tensor(
            prod[:, HF:CH], tbs[:, HF:CH], iot[:, HF:CH], op=mybir.AluOpType.mult
        )
        nc.vector.tensor_reduce(
            outb[:, 2:3], prod[:, HF:CH], axis=mybir.AxisListType.X,
            op=mybir.AluOpType.max,
        ).then_inc(dn, 1)

        # Output DMAs split across both HWDGE queues (parallel fetch).
        nc.sync.wait_ge(dn, 4)
        nc.sync.dma_start(out[0:HP, :], outb[0:HP, :]).then_inc(dn, 16)
        nc.scalar.wait_ge(dn, 4)
        nc.scalar.dma_start(out[HP:P, :], outb[HP:P, :]).then_inc(dn, 16)
        if final_wait:
            nc.sync.wait_ge(dn, 36)

    return nc


def _get_nc(**kwargs):
    key = tuple(sorted(kwargs.items()))
    if key not in _CACHE:
        _CACHE[key] = _build_nc(**kwargs)
    return _CACHE[key]


def _in_maps(trg_boundary: np.ndarray):
    tb = np.asarray(trg_boundary)
    assert tb.shape == (B, T), tb.shape
    tb8 = tb.astype(np.int8)  # values are 0/1
    P = BS * NCHUNK
    return [
        {"tb": tb8[c * BS : (c + 1) * BS].reshape(P, CH)}
        for c in range(N_CORES)
    ]


def run_device(trg_boundary, nc_kwargs=None, **run_kwargs):
    """Compile (cached) + run on cores 0-7; returns BassKernelResults."""
    from concourse.bass_utils import run_bass_kernel_spmd

    return run_bass_kernel_spmd(
        _get_nc(**(nc_kwargs or {})),
        _in_maps(trg_boundary),
        core_ids=list(range(N_CORES)),
        **run_kwargs,
    )


def kernel(src_sent, trg_sent, src_boundary, trg_boundary):
    res = run_device(trg_boundary)
    tb = np.asarray(trg_boundary)
    off = np.arange(NCHUNK, dtype=np.float64) * CH  # chunk base offsets
    total = np.float64(0.0)
    for c, r in enumerate(res.results):
        o = np.asarray(r["out"], dtype=np.float64)  # [128, 4]
        m = np.maximum(
            np.where(o[:, 0] > 0, o[:, 0], -1.0),
            np.where(o[:, 2] > 0, o[:, 2], -1.0),
        )  # per-partition lastp1 (local, 1..512); -1 if chunk empty
        cnt = (o[:, 1] + o[:, 3]).reshape(BS, NCHUNK)
        m4 = m.reshape(BS, NCHUNK)
        lastp1 = np.where(m4 > 0, m4 + off, 0.0).max(axis=1)
        count = cnt.sum(axis=1)
        first = tb[c * BS : (c + 1) * BS, 0].astype(np.float64)
        sum_scales = count - first - np.maximum(lastp1, 1.0) + T + 1
        total += sum_scales.sum()
    return np.asarray(total * NEG_LOG_K, dtype=np.float32)
